# revision 4
# baseline (speedup 1.0000x reference)
"""FRFN forward kernel for 8 Trainium2 NeuronCores.

Sharding: pure data parallel over batch B=64 -> 8 batches per core.
The TVConv generated weight (batch-independent) is recomputed on every
core.

Per-core pipeline (channel dims padded so x1/x2 halves align at 768):
  weightgen: 3x (3x3 conv + LayerNorm(CHW) + relu) on 4x14x14 posi map
             then final conv -> wgt[ct] = (128, 9, 196) bf16 per tile
  proj_in  : h = W_in @ x        PE bf16 -> padded (128, 8, 16, 16)
  tvconv   : prod_k = wgt_k * h_win_k   DVE (6 taps) + Pool (3 taps)
             partial pre-adds on DVE, remaining streams summed on the
             PE as identity-matmul PSUM accumulation
  gate     : x1 tiles: gelu straight from PSUM (ACT)
             x2 tiles: gated = ga * psum  (DVE, mixed dtype)
  proj_out : W_out @ gated               PE bf16
"""

import numpy as np
import ml_dtypes
from contextlib import ExitStack

import concourse.bacc as bacc
import concourse.bass as bass
import concourse.mybir as mybir
import concourse.tile as tile
from concourse.bass_utils import run_bass_kernel_spmd

F32 = mybir.dt.float32
BF16 = mybir.dt.bfloat16
AF = mybir.ActivationFunctionType
OP = mybir.AluOpType

NCORES = 8
B = 64
BPC = B // NCORES          # 8 batches per core
DIM = 256
HID = 680
CH = 2 * HID               # 1360
HIDP = 768                 # padded x1/x2 half (6 * 128)
CHP = 2 * HIDP             # 1536
NCT = CHP // 128           # 12 channel tiles
NGT = HIDP // 128          # 6 gate tiles
HP = 14
NIJ = HP * HP              # 196
PH = 16                    # padded spatial side
INTER = 64
NKPL = 9                   # 3x3 taps
KT_ROWS = [128, 128, 128, 128, 64]   # 576 contraction rows
NCHUNK = 4                 # 392-col psum chunks (2 batches x 196)
NB2 = 2 * NIJ              # 392
EPS = 1e-5
NLN = float(INTER * NIJ)

# channel-tile visit order: gate pairs adjacent (x1 then its x2)
CT_ORDER = [0, 6, 1, 7, 2, 8, 3, 9, 4, 10, 5, 11]
# taps computed on the Pool engine (slow, so they stream last)
POOL_TAPS = (6, 7, 8)
# pre-add pairs on DVE (merged before the PE identity-sum)
PRE_A3 = [(0, 1), (2, 3), (4, 5)]           # streams: 0,2,4,6,7,8
PRE_A4 = [(0, 1), (2, 3), (4, 5), (6, 7)]   # streams: 0,2,4,6,8

_CACHE = {}


def _build_nc():
    nc = bacc.Bacc("TRN2", target_bir_lowering=False)

    xT = nc.dram_tensor("xT", [DIM, BPC * NIJ], BF16, kind="ExternalInput")
    winT = nc.dram_tensor("winT", [DIM, CHP], BF16, kind="ExternalInput")
    posiP = nc.dram_tensor("posiP", [4, PH, PH], BF16, kind="ExternalInput")
    w0T = nc.dram_tensor("w0T", [4, NKPL, INTER], BF16, kind="ExternalInput")
    w1T = nc.dram_tensor("w1T", [INTER, NKPL, INTER], BF16,
                         kind="ExternalInput")
    w2T = nc.dram_tensor("w2T", [INTER, NKPL, INTER], BF16,
                         kind="ExternalInput")
    gb = nc.dram_tensor("gb", [INTER, 6, NIJ], F32, kind="ExternalInput")
    wfT = nc.dram_tensor("wfT", [576, NKPL * CHP], BF16, kind="ExternalInput")
    woutT = nc.dram_tensor("woutT", [HIDP, DIM], BF16, kind="ExternalInput")
    identD = nc.dram_tensor("identD", [128, 128], BF16, kind="ExternalInput")
    out_f = nc.dram_tensor("out_f", [DIM, BPC * NIJ], F32,
                           kind="ExternalOutput")

    with tile.TileContext(nc) as tc, ExitStack() as ctx:
        persist = ctx.enter_context(tc.tile_pool(name="persist", bufs=1))
        work = ctx.enter_context(tc.tile_pool(name="work", bufs=2))
        wgtpool = ctx.enter_context(tc.tile_pool(name="wgtpool", bufs=2))
        prodpool = ctx.enter_context(tc.tile_pool(name="prodpool", bufs=1))
        gapool = ctx.enter_context(tc.tile_pool(name="gapool", bufs=2))
        wfpool = ctx.enter_context(tc.tile_pool(name="wfpool", bufs=2))
        # 4 rolling single-bank psum units + one 4-bank accumulator
        ps_u = ctx.enter_context(
            tc.tile_pool(name="ps_u", bufs=4, space="PSUM"))
        ps_pst = ctx.enter_context(
            tc.tile_pool(name="ps_pst", bufs=1, space="PSUM"))

        # ---------------- persistent SBUF tensors ----------------
        h_sb = [persist.tile([128, BPC, PH, PH], BF16, name="t", tag=f"h{i}")
                for i in range(NCT)]
        gated = [persist.tile([128, BPC * NIJ], BF16, name="t", tag=f"gd{i}")
                 for i in range(NGT)]
        wout_sb = [persist.tile([128, DIM], BF16, name="t", tag=f"wo{i}")
                   for i in range(NGT)]
        x_sb = [persist.tile([128, BPC * NIJ], BF16, name="t", tag=f"x{i}")
                for i in range(2)]
        win_sb = [persist.tile([128, CHP], BF16, name="t", tag=f"wi{i}")
                  for i in range(2)]

        posi_sb = persist.tile([4, PH, PH], BF16, name="t", tag="posi")
        w0_sb = persist.tile([4, NKPL, INTER], BF16, name="t", tag="w0")
        w1_sb = persist.tile([INTER, NKPL, INTER], BF16, name="t", tag="w1")
        w2_sb = persist.tile([INTER, NKPL, INTER], BF16, name="t", tag="w2")
        gb_sb = persist.tile([INTER, 6, NIJ], F32, name="t", tag="gb")
        pad1 = persist.tile([INTER, PH, PH], BF16, name="t", tag="pad1")
        pad2 = persist.tile([INTER, PH, PH], BF16, name="t", tag="pad2")
        pad3 = persist.tile([INTER, PH, PH], BF16, name="t", tag="pad3")
        p3 = [persist.tile([KT_ROWS[k], NIJ], BF16, name="t", tag=f"p3_{k}")
              for k in range(5)]
        ones_c = persist.tile([INTER, 1], F32, name="t", tag="ones_c")
        ones_r = persist.tile([1, INTER], F32, name="t", tag="ones_r")
        ident = persist.tile([128, 128], BF16, name="t", tag="ident")
        eps_t = persist.tile([1, 1], F32, name="t", tag="eps")

        # ---------------- input DMAs + memsets ----------------
        nc.sync.dma_start(posi_sb[:], posiP[:])
        nc.sync.dma_start(ident[:], identD[:])
        nc.sync.dma_start(w0_sb[:], w0T[:])
        nc.sync.dma_start(w1_sb[:], w1T[:])
        nc.sync.dma_start(w2_sb[:], w2T[:])
        nc.sync.dma_start(gb_sb[:], gb[:])
        for i in range(2):
            nc.sync.dma_start(x_sb[i][:], xT[128 * i:128 * (i + 1), :])
            nc.sync.dma_start(win_sb[i][:], winT[128 * i:128 * (i + 1), :])
        for i in range(NGT):
            nc.sync.dma_start(wout_sb[i][:], woutT[128 * i:128 * (i + 1), :])

        # pre-warm the sqrt ACT table (LN chain); the gelu table is warmed
        # by a dummy activation right after the chain so the 1.3us table
        # load lands off the critical path
        warm = persist.tile([1, 1], F32, name="t", tag="warm")
        wsink = persist.tile([1, 1], F32, name="t", tag="wsink")
        nc.gpsimd.memset(warm[:], 1.0)
        nc.scalar.activation(wsink[:], warm[:], AF.Sqrt)

        nc.gpsimd.memset(ones_c[:], 1.0)
        nc.gpsimd.memset(eps_t[:], EPS)
        nc.gpsimd.memset(ones_r[:], 1.0)
        nc.gpsimd.memset(pad1[:], 0.0)
        nc.gpsimd.memset(pad2[:], 0.0)
        nc.gpsimd.memset(pad3[:], 0.0)
        for i in range(NCT):
            # zero only the pad borders (proj_in drains fill the interior);
            # on DVE, which is otherwise idle during the head
            t = h_sb[i]
            nc.vector.memset(t[:, :, 0, :], 0.0)
            nc.vector.memset(t[:, :, 15, :], 0.0)
            nc.vector.memset(t[:, :, 1:15, 0], 0.0)
            nc.vector.memset(t[:, :, 1:15, 15], 0.0)

        # ------------- weight-gen small conv chain (fp32) -------------
        def layernorm_relu(ps_in, g_ap, b_ap, pad_tile):
            sq = work.tile([INTER, NIJ], F32, name="t", tag="ln_sq")
            hval = work.tile([INTER, NIJ], F32, name="t", tag="ln_h")
            stats = work.tile([INTER, 2], F32, name="t", tag="ln_st")
            nc.scalar.activation(sq[:], ps_in, AF.Square,
                                 accum_out=stats[:, 1:2])
            nc.scalar.activation(hval[:], ps_in, AF.Copy,
                                 accum_out=stats[:, 0:1])
            ps_r = ps_u.tile([1, 2], F32, name="t", tag="u")
            nc.tensor.matmul(ps_r[:], ones_c[:], stats[:],
                             start=True, stop=True)
            mr = work.tile([1, 2], F32, name="t", tag="ln_mr")
            musq = work.tile([1, 1], F32, name="t", tag="ln_musq")
            nc.scalar.activation(musq[:], ps_r[:, 0:1], AF.Square,
                                 scale=1.0 / NLN)
            e2e = work.tile([1, 1], F32, name="t", tag="ln_e2e")
            nc.scalar.activation(e2e[:], ps_r[:, 1:2], AF.Identity,
                                 scale=1.0 / NLN, bias=eps_t[:])
            nc.scalar.activation(mr[:, 0:1], ps_r[:, 0:1], AF.Copy,
                                 scale=1.0 / NLN)
            std = work.tile([1, 1], F32, name="t", tag="ln_std")
            nc.scalar.activation(std[:], musq[:], AF.Sqrt,
                                 scale=-1.0, bias=e2e[:])
            nc.vector.reciprocal(mr[:, 1:2], std[:])
            ps_bc = ps_u.tile([INTER, 2], F32, name="t", tag="u")
            nc.tensor.matmul(ps_bc[:], ones_r[:], mr[:], start=True, stop=True)
            bc = work.tile([INTER, 2], F32, name="t", tag="ln_bc")
            nc.scalar.activation(bc[:], ps_bc[:], AF.Copy)
            xn = work.tile([INTER, NIJ], F32, name="t", tag="ln_xn")
            nc.vector.tensor_scalar(xn[:], hval[:], bc[:, 0:1], bc[:, 1:2],
                                    op0=OP.subtract, op1=OP.mult)
            t2 = work.tile([INTER, NIJ], F32, name="t", tag="ln_t2")
            nc.vector.tensor_mul(t2[:], xn[:], g_ap)
            t3 = work.tile([INTER, NIJ], F32, name="t", tag="ln_t3")
            nc.vector.tensor_add(t3[:], t2[:], b_ap)
            dst = pad_tile[:, 1:15, 1:15]
            src = t3[:].rearrange("p (i j) -> p i j", i=HP, j=HP)
            nc.scalar.activation(dst, src, AF.Relu)

        def conv3x3(w_sb, pad_tile, ps_out):
            for kap in range(NKPL):
                di, dj = kap // 3, kap % 3
                nc.tensor.matmul(ps_out, w_sb[:, kap, :],
                                 pad_tile[:, di:di + HP, dj:dj + HP],
                                 start=(kap == 0), stop=(kap == NKPL - 1))

        # ---------------- per-stage emission helpers ----------------
        def emit_proj_in(ct):
            """proj_in for one channel tile -> h_sb[ct] (padded layout)."""
            for ch in range(NCHUNK):
                u = ps_u.tile([128, NB2], F32, name="t", tag="u")
                for kt in range(2):
                    nc.tensor.matmul(
                        u[:],
                        win_sb[kt][:, 128 * ct:128 * (ct + 1)],
                        x_sb[kt][:, NB2 * ch:NB2 * (ch + 1)],
                        start=(kt == 0), stop=(kt == 1))
                dst = h_sb[ct][:, 2 * ch:2 * ch + 2, 1:15, 1:15]
                src = u[:].rearrange("p (b i j) -> p b i j", b=2, i=HP, j=HP)
                nc.scalar.activation(dst, src, AF.Copy)

        def emit_wf_load(ct):
            wf_t = []
            r0 = 0
            c0 = NKPL * 128 * ct
            for kt in range(5):
                t = wfpool.tile([KT_ROWS[kt], NKPL * 128], BF16,
                                name="t", tag=f"wf{kt}")
                nc.sync.dma_start(
                    t[:], wfT[r0:r0 + KT_ROWS[kt], c0:c0 + NKPL * 128])
                wf_t.append(t)
                r0 += KT_ROWS[kt]
            return wf_t

        def emit_conv_f(ct, wf_t):
            """final conv for one channel tile -> wgt (128, 9, 196) bf16.
            Two taps share one psum bank; drained in 2-tap batches."""
            wgt = wgtpool.tile([128, NKPL, NIJ], BF16, name="t", tag="wgt")
            for t0 in range(0, NKPL, 2):
                ntap = min(2, NKPL - t0)
                u = ps_u.tile([128, NB2], F32, name="t", tag="u")
                for sub in range(ntap):
                    kpl = t0 + sub
                    dst = u[:, NIJ * sub:NIJ * (sub + 1)]
                    for kt in range(5):
                        nc.tensor.matmul(
                            dst,
                            wf_t[kt][:, 128 * kpl:128 * (kpl + 1)],
                            p3[kt][:],
                            start=(kt == 0), stop=(kt == 4))
                nc.scalar.activation(
                    wgt[:, t0:t0 + ntap, :],
                    u[:, 0:NIJ * ntap].rearrange("p (t f) -> p t f", t=ntap),
                    AF.Copy)
            return wgt

        def emit_taps(ct, wgt, pst, preadds):
            """9 tap products (DVE+Pool), pre-adds, identity-sum into pst."""
            prods = {}
            for t in range(NKPL):
                di, dj = t // 3, t % 3
                wgb = (wgt[:, t, :].rearrange("p (i j) -> p i j", i=HP, j=HP)
                       .unsqueeze(1).broadcast_to((128, BPC, HP, HP)))
                hwin = h_sb[ct][:, :, di:di + HP, dj:dj + HP]
                prod = prodpool.tile([128, BPC * NIJ], BF16,
                                     name="t", tag=f"prod{t}")
                pr = prod[:].rearrange("p (b i j) -> p b i j",
                                       b=BPC, i=HP, j=HP)
                eng = nc.gpsimd if t in POOL_TAPS else nc.vector
                eng.tensor_mul(pr, hwin, wgb)
                prods[t] = prod

            merged = set()
            for (a, b) in preadds:
                nc.vector.tensor_add(prods[a][:], prods[a][:], prods[b][:])
                merged.add(b)
            streams = [prods[t] for t in range(NKPL) if t not in merged]

            for si, prod in enumerate(streams):
                for ch in range(NCHUNK):
                    nc.tensor.matmul(
                        pst[:, ch, 0:NB2], ident[:],
                        prod[:, NB2 * ch:NB2 * (ch + 1)],
                        start=(si == 0), stop=(si == len(streams) - 1))

        # =================== program ===================
        ps0 = ps_u.tile([INTER, NIJ], F32, name="t", tag="u")
        conv3x3(w0_sb, posi_sb, ps0[:])
        layernorm_relu(ps0[:], gb_sb[:, 0, :], gb_sb[:, 1, :], pad1)

        # proj_in prologue interleaved with the LN chain
        emit_proj_in(CT_ORDER[0])

        ps1 = ps_u.tile([INTER, NIJ], F32, name="t", tag="u")
        conv3x3(w1_sb, pad1, ps1[:])
        layernorm_relu(ps1[:], gb_sb[:, 2, :], gb_sb[:, 3, :], pad2)

        emit_proj_in(CT_ORDER[1])

        ps2 = ps_u.tile([INTER, NIJ], F32, name="t", tag="u")
        conv3x3(w2_sb, pad2, ps2[:])
        layernorm_relu(ps2[:], gb_sb[:, 4, :], gb_sb[:, 5, :], pad3)

        # warm the gelu table now (off the critical path)
        nc.scalar.activation(wsink[:], warm[:], AF.Gelu)

        # im2col of pad3 for the final conv (576 contraction rows)
        qengs = [nc.sync, nc.scalar]
        for kt in range(5):
            nk = KT_ROWS[kt] // 64
            for sub in range(nk):
                kap = 2 * kt + sub
                di, dj = kap // 3, kap % 3
                srcw = pad3[:, di:di + HP, dj:dj + HP]
                dst = p3[kt][64 * sub:64 * (sub + 1), :]
                dst = dst.rearrange("p (i j) -> p i j", i=HP, j=HP)
                qengs[kap % 2].dma_start(dst, srcw)

        emit_proj_in(CT_ORDER[2])

        wf_next = emit_wf_load(CT_ORDER[0])
        ga_cur = None
        for s, ct in enumerate(CT_ORDER):
            wf_t = wf_next
            wgt = emit_conv_f(ct, wf_t)
            if s + 1 < NCT:
                wf_next = emit_wf_load(CT_ORDER[s + 1])
            if s + 3 < NCT:
                emit_proj_in(CT_ORDER[s + 3])

            pst = ps_pst.tile([128, NCHUNK, 512], F32, name="t", tag="pst")
            preadds = PRE_A3 if s % 2 == 0 else PRE_A4
            emit_taps(ct, wgt, pst, preadds)

            if s % 2 == 0:
                # x1 tile: gelu straight from psum
                ga = gapool.tile([128, BPC * NIJ], BF16, name="t", tag="ga")
                nc.scalar.activation(
                    ga[:].rearrange("p (c f) -> p c f", c=NCHUNK),
                    pst[:, :, 0:NB2], AF.Gelu)
                ga_cur = ga
            else:
                # x2 tile: gated = gelu(x1) * x2  (psum f32 * sbuf bf16)
                pair = ct - NGT
                nc.vector.tensor_mul(
                    gated[pair][:].rearrange("p (c f) -> p c f", c=NCHUNK),
                    pst[:, :, 0:NB2],
                    ga_cur[:].rearrange("p (c f) -> p c f", c=NCHUNK))

        # ---------------- proj_out: W_out @ gated ----------------
        outpool = ctx.enter_context(tc.tile_pool(name="outpool", bufs=4))
        for m in range(2):
            for ch in range(NCHUNK):
                u = ps_u.tile([128, NB2], F32, name="t", tag="u")
                for kt in range(NGT):
                    nc.tensor.matmul(
                        u[:],
                        wout_sb[kt][:, 128 * m:128 * (m + 1)],
                        gated[kt][:, NB2 * ch:NB2 * (ch + 1)],
                        start=(kt == 0), stop=(kt == NGT - 1))
                ot = outpool.tile([128, NB2], F32, name="t", tag="ot")
                nc.scalar.activation(ot[:], u[:], AF.Copy)
                nc.sync.dma_start(
                    out_f[128 * m:128 * (m + 1), NB2 * ch:NB2 * (ch + 1)],
                    ot[:])

    nc.compile()
    return nc


def _pack_shared(inputs):
    """Pack the batch-independent tensors (host-side layout marshalling)."""
    W_in = np.asarray(inputs["W_in"], np.float32)
    W_out = np.asarray(inputs["W_out"], np.float32)
    posi = np.asarray(inputs["posi_map"], np.float32)
    w0 = np.asarray(inputs["w0"], np.float32)
    w1 = np.asarray(inputs["w1"], np.float32)
    w2 = np.asarray(inputs["w2"], np.float32)
    wf = np.asarray(inputs["wf"], np.float32)

    padc = np.arange(CH)
    padc = np.where(padc < HID, padc, padc + (HIDP - HID))

    winP = np.zeros((CHP, DIM), np.float32)
    winP[padc] = W_in
    winT = np.ascontiguousarray(winP.T).astype(ml_dtypes.bfloat16)

    w0T = np.ascontiguousarray(
        w0.transpose(1, 2, 3, 0).reshape(4, 9, INTER)).astype(ml_dtypes.bfloat16)
    w1T = np.ascontiguousarray(
        w1.transpose(1, 2, 3, 0).reshape(INTER, 9, INTER)
    ).astype(ml_dtypes.bfloat16)
    w2T = np.ascontiguousarray(
        w2.transpose(1, 2, 3, 0).reshape(INTER, 9, INTER)
    ).astype(ml_dtypes.bfloat16)

    posiP = np.zeros((4, PH, PH), np.float32)
    posiP[:, 1:15, 1:15] = posi[0]
    posiP = posiP.astype(ml_dtypes.bfloat16)

    gbs = [np.asarray(inputs[k], np.float32).reshape(INTER, NIJ)
           for k in ("g0", "b0", "g1", "b1", "g2", "b2")]
    gb = np.stack(gbs, axis=1)   # (64, 6, 196)

    # wfT[(kh,kw,cin) row, ct*1152 + kpl*128 + p] = wf[c*9+kpl, cin, kh, kw]
    wf5 = wf.reshape(CH, NKPL, INTER, 3, 3)
    wf5 = wf5.transpose(3, 4, 2, 1, 0)          # (kh, kw, cin, kpl, c)
    wfTp = np.zeros((576, NKPL, CHP), np.float32)
    wfTp[:, :, padc] = wf5.reshape(576, NKPL, CH)
    wfTp = wfTp.reshape(576, NKPL, NCT, 128).transpose(0, 2, 1, 3)
    wfT = np.ascontiguousarray(
        wfTp.reshape(576, NKPL * CHP)).astype(ml_dtypes.bfloat16)

    woP = np.zeros((HIDP, DIM), np.float32)
    woP[:HID] = W_out.T
    woutT = woP.astype(ml_dtypes.bfloat16)

    return dict(winT=winT, posiP=posiP, w0T=w0T, w1T=w1T, w2T=w2T,
                gb=np.ascontiguousarray(gb), wfT=wfT, woutT=woutT,
                identD=np.eye(128, dtype=ml_dtypes.bfloat16))


def kernel(**inputs) -> np.ndarray:
    if "nc" not in _CACHE:
        _CACHE["nc"] = _build_nc()
    nc = _CACHE["nc"]

    x = np.asarray(inputs["x"], np.float32)     # (64, 256, 14, 14)
    shared = _pack_shared(inputs)

    in_maps = []
    for c in range(NCORES):
        xc = x[BPC * c:BPC * (c + 1)]           # (8, 256, 14, 14)
        xT = np.ascontiguousarray(
            xc.transpose(1, 0, 2, 3).reshape(DIM, BPC * NIJ)
        ).astype(ml_dtypes.bfloat16)
        m = dict(shared)
        m["xT"] = xT
        in_maps.append(m)

    res = run_bass_kernel_spmd(nc, in_maps, list(range(NCORES)))
    outs = []
    for c in range(NCORES):
        o = res.results[c]["out_f"].reshape(DIM, BPC, HP, HP)
        outs.append(o.transpose(1, 0, 2, 3))
    return np.ascontiguousarray(np.concatenate(outs, axis=0), dtype=np.float32)


# revision 5
# speedup vs baseline: 1.0090x; 1.0090x over previous
"""FRFN forward kernel for 8 Trainium2 NeuronCores.

Sharding: pure data parallel over batch B=64 -> 8 batches per core.
The TVConv generated weight (batch-independent) is recomputed on every
core.

Per-core pipeline (channel dims padded so x1/x2 halves align at 768):
  weightgen: 3x (3x3 conv + LayerNorm(CHW) + relu) on 4x14x14 posi map
             then final conv -> wgt[ct] = (128, 9, 196) bf16 per tile
  proj_in  : h = W_in @ x        PE bf16 -> padded (128, 8, 16, 16)
  tvconv   : prod_k = wgt_k * h_win_k   DVE (6 taps) + Pool (3 taps)
             partial pre-adds on DVE, remaining streams summed on the
             PE as identity-matmul PSUM accumulation
  gate     : x1 tiles: gelu straight from PSUM (ACT)
             x2 tiles: gated = ga * psum  (DVE, mixed dtype)
  proj_out : W_out @ gated               PE bf16
"""

import numpy as np
import ml_dtypes
from contextlib import ExitStack

import concourse.bacc as bacc
import concourse.bass as bass
import concourse.mybir as mybir
import concourse.tile as tile
from concourse.bass_utils import run_bass_kernel_spmd

F32 = mybir.dt.float32
BF16 = mybir.dt.bfloat16
AF = mybir.ActivationFunctionType
OP = mybir.AluOpType

NCORES = 8
B = 64
BPC = B // NCORES          # 8 batches per core
DIM = 256
HID = 680
CH = 2 * HID               # 1360
HIDP = 768                 # padded x1/x2 half (6 * 128)
CHP = 2 * HIDP             # 1536
NCT = CHP // 128           # 12 channel tiles
NGT = HIDP // 128          # 6 gate tiles
HP = 14
NIJ = HP * HP              # 196
PH = 16                    # padded spatial side
INTER = 64
NKPL = 9                   # 3x3 taps
KT_ROWS = [128, 128, 128, 128, 64]   # 576 contraction rows
NCHUNK = 4                 # 392-col psum chunks (2 batches x 196)
NB2 = 2 * NIJ              # 392
EPS = 1e-5
NLN = float(INTER * NIJ)

# channel-tile visit order: gate pairs adjacent (x1 then its x2)
CT_ORDER = [0, 6, 1, 7, 2, 8, 3, 9, 4, 10, 5, 11]
# taps computed on the Pool engine (slow, so they stream last)
POOL_TAPS = (6, 7, 8)
# pre-add pairs on DVE (merged before the PE identity-sum)
PRE_A3 = [(0, 1), (2, 3), (4, 5)]           # streams: 0,2,4,6,7,8
PRE_A4 = [(0, 1), (2, 3), (4, 5), (6, 7)]   # streams: 0,2,4,6,8

_CACHE = {}


def _build_nc():
    nc = bacc.Bacc("TRN2", target_bir_lowering=False)

    xT = nc.dram_tensor("xT", [DIM, BPC * NIJ], BF16, kind="ExternalInput")
    winT = nc.dram_tensor("winT", [DIM, CHP], BF16, kind="ExternalInput")
    posiP = nc.dram_tensor("posiP", [4, PH, PH], BF16, kind="ExternalInput")
    w0T = nc.dram_tensor("w0T", [4, NKPL, INTER], BF16, kind="ExternalInput")
    w1T = nc.dram_tensor("w1T", [INTER, NKPL, INTER], BF16,
                         kind="ExternalInput")
    w2T = nc.dram_tensor("w2T", [INTER, NKPL, INTER], BF16,
                         kind="ExternalInput")
    gb = nc.dram_tensor("gb", [INTER, 6, NIJ], F32, kind="ExternalInput")
    wfT = nc.dram_tensor("wfT", [576, NKPL * CHP], BF16, kind="ExternalInput")
    woutT = nc.dram_tensor("woutT", [HIDP, DIM], BF16, kind="ExternalInput")
    identD = nc.dram_tensor("identD", [128, 128], BF16, kind="ExternalInput")
    out_f = nc.dram_tensor("out_f", [DIM, BPC * NIJ], F32,
                           kind="ExternalOutput")

    with tile.TileContext(nc) as tc, ExitStack() as ctx:
        persist = ctx.enter_context(tc.tile_pool(name="persist", bufs=1))
        work = ctx.enter_context(tc.tile_pool(name="work", bufs=2))
        wgtpool = ctx.enter_context(tc.tile_pool(name="wgtpool", bufs=2))
        prodpool = ctx.enter_context(tc.tile_pool(name="prodpool", bufs=1))
        gapool = ctx.enter_context(tc.tile_pool(name="gapool", bufs=2))
        wfpool = ctx.enter_context(tc.tile_pool(name="wfpool", bufs=2))
        # 4 rolling single-bank psum units + one 4-bank accumulator
        ps_u = ctx.enter_context(
            tc.tile_pool(name="ps_u", bufs=4, space="PSUM"))
        ps_pst = ctx.enter_context(
            tc.tile_pool(name="ps_pst", bufs=1, space="PSUM"))

        # ---------------- persistent SBUF tensors ----------------
        h_sb = [persist.tile([128, BPC, PH, PH], BF16, name="t", tag=f"h{i}")
                for i in range(NCT)]
        gated = [persist.tile([128, BPC * NIJ], BF16, name="t", tag=f"gd{i}")
                 for i in range(NGT)]
        wout_sb = [persist.tile([128, DIM], BF16, name="t", tag=f"wo{i}")
                   for i in range(NGT)]
        x_sb = [persist.tile([128, BPC * NIJ], BF16, name="t", tag=f"x{i}")
                for i in range(2)]
        win_sb = [persist.tile([128, CHP], BF16, name="t", tag=f"wi{i}")
                  for i in range(2)]

        posi_sb = persist.tile([4, PH, PH], BF16, name="t", tag="posi")
        w0_sb = persist.tile([4, NKPL, INTER], BF16, name="t", tag="w0")
        w1_sb = persist.tile([INTER, NKPL, INTER], BF16, name="t", tag="w1")
        w2_sb = persist.tile([INTER, NKPL, INTER], BF16, name="t", tag="w2")
        gb_sb = persist.tile([INTER, 6, NIJ], F32, name="t", tag="gb")
        pad1 = persist.tile([INTER, PH, PH], BF16, name="t", tag="pad1")
        pad2 = persist.tile([INTER, PH, PH], BF16, name="t", tag="pad2")
        pad3 = persist.tile([INTER, PH, PH], BF16, name="t", tag="pad3")
        p3 = [persist.tile([KT_ROWS[k], NIJ], BF16, name="t", tag=f"p3_{k}")
              for k in range(5)]
        ones_c = persist.tile([INTER, 1], F32, name="t", tag="ones_c")
        ones_r = persist.tile([1, INTER], F32, name="t", tag="ones_r")
        ident = persist.tile([128, 128], BF16, name="t", tag="ident")
        eps_t = persist.tile([1, 1], F32, name="t", tag="eps")

        # ---------------- input DMAs + memsets ----------------
        nc.sync.dma_start(posi_sb[:], posiP[:])
        nc.sync.dma_start(ident[:], identD[:])
        nc.sync.dma_start(w0_sb[:], w0T[:])
        nc.sync.dma_start(w1_sb[:], w1T[:])
        nc.sync.dma_start(w2_sb[:], w2T[:])
        nc.sync.dma_start(gb_sb[:], gb[:])
        for i in range(2):
            nc.sync.dma_start(x_sb[i][:], xT[128 * i:128 * (i + 1), :])
            nc.sync.dma_start(win_sb[i][:], winT[128 * i:128 * (i + 1), :])
        for i in range(NGT):
            nc.sync.dma_start(wout_sb[i][:], woutT[128 * i:128 * (i + 1), :])

        # pre-warm the sqrt ACT table (LN chain); the gelu table is warmed
        # by a dummy activation right after the chain so the 1.3us table
        # load lands off the critical path
        warm = persist.tile([1, 1], F32, name="t", tag="warm")
        wsink = persist.tile([1, 1], F32, name="t", tag="wsink")
        nc.gpsimd.memset(warm[:], 1.0)
        nc.scalar.activation(wsink[:], warm[:], AF.Sqrt)

        nc.gpsimd.memset(ones_c[:], 1.0)
        nc.gpsimd.memset(eps_t[:], EPS)
        nc.gpsimd.memset(ones_r[:], 1.0)
        nc.gpsimd.memset(pad1[:], 0.0)
        nc.gpsimd.memset(pad2[:], 0.0)
        nc.gpsimd.memset(pad3[:], 0.0)
        for i in range(NCT):
            # zero only the pad borders (proj_in drains fill the interior);
            # on DVE, which is otherwise idle during the head
            t = h_sb[i]
            nc.vector.memset(t[:, :, 0, :], 0.0)
            nc.vector.memset(t[:, :, 15, :], 0.0)
            nc.vector.memset(t[:, :, 1:15, 0], 0.0)
            nc.vector.memset(t[:, :, 1:15, 15], 0.0)

        # ------------- weight-gen small conv chain (fp32) -------------
        def layernorm_relu(ps_in, g_ap, b_ap, pad_tile):
            sq = work.tile([INTER, NIJ], F32, name="t", tag="ln_sq")
            hval = work.tile([INTER, NIJ], F32, name="t", tag="ln_h")
            stats = work.tile([INTER, 2], F32, name="t", tag="ln_st")
            nc.scalar.activation(sq[:], ps_in, AF.Square,
                                 accum_out=stats[:, 1:2])
            nc.scalar.activation(hval[:], ps_in, AF.Copy,
                                 accum_out=stats[:, 0:1])
            ps_r = ps_u.tile([1, 2], F32, name="t", tag="u")
            nc.tensor.matmul(ps_r[:], ones_c[:], stats[:],
                             start=True, stop=True)
            mr = work.tile([1, 2], F32, name="t", tag="ln_mr")
            musq = work.tile([1, 1], F32, name="t", tag="ln_musq")
            nc.scalar.activation(musq[:], ps_r[:, 0:1], AF.Square,
                                 scale=1.0 / NLN)
            e2e = work.tile([1, 1], F32, name="t", tag="ln_e2e")
            nc.scalar.activation(e2e[:], ps_r[:, 1:2], AF.Identity,
                                 scale=1.0 / NLN, bias=eps_t[:])
            nc.scalar.activation(mr[:, 0:1], ps_r[:, 0:1], AF.Copy,
                                 scale=1.0 / NLN)
            std = work.tile([1, 1], F32, name="t", tag="ln_std")
            nc.scalar.activation(std[:], musq[:], AF.Sqrt,
                                 scale=-1.0, bias=e2e[:])
            nc.vector.reciprocal(mr[:, 1:2], std[:])
            ps_bc = ps_u.tile([INTER, 2], F32, name="t", tag="u")
            nc.tensor.matmul(ps_bc[:], ones_r[:], mr[:], start=True, stop=True)
            bc = work.tile([INTER, 2], F32, name="t", tag="ln_bc")
            nc.scalar.activation(bc[:], ps_bc[:], AF.Copy)
            xn = work.tile([INTER, NIJ], F32, name="t", tag="ln_xn")
            nc.vector.tensor_scalar(xn[:], hval[:], bc[:, 0:1], bc[:, 1:2],
                                    op0=OP.subtract, op1=OP.mult)
            t2 = work.tile([INTER, NIJ], F32, name="t", tag="ln_t2")
            nc.vector.tensor_mul(t2[:], xn[:], g_ap)
            t3 = work.tile([INTER, NIJ], F32, name="t", tag="ln_t3")
            nc.vector.tensor_add(t3[:], t2[:], b_ap)
            dst = pad_tile[:, 1:15, 1:15]
            src = t3[:].rearrange("p (i j) -> p i j", i=HP, j=HP)
            nc.scalar.activation(dst, src, AF.Relu)

        def conv3x3(w_sb, pad_tile, ps_out):
            for kap in range(NKPL):
                di, dj = kap // 3, kap % 3
                nc.tensor.matmul(ps_out, w_sb[:, kap, :],
                                 pad_tile[:, di:di + HP, dj:dj + HP],
                                 start=(kap == 0), stop=(kap == NKPL - 1))

        # ---------------- per-stage emission helpers ----------------
        def emit_proj_in(ct):
            """proj_in for one channel tile -> h_sb[ct] (padded layout)."""
            for ch in range(NCHUNK):
                u = ps_u.tile([128, NB2], F32, name="t", tag="u")
                for kt in range(2):
                    nc.tensor.matmul(
                        u[:],
                        win_sb[kt][:, 128 * ct:128 * (ct + 1)],
                        x_sb[kt][:, NB2 * ch:NB2 * (ch + 1)],
                        start=(kt == 0), stop=(kt == 1))
                dst = h_sb[ct][:, 2 * ch:2 * ch + 2, 1:15, 1:15]
                src = u[:].rearrange("p (b i j) -> p b i j", b=2, i=HP, j=HP)
                nc.scalar.activation(dst, src, AF.Copy)

        def emit_wf_load(ct):
            wf_t = []
            r0 = 0
            c0 = NKPL * 128 * ct
            for kt in range(5):
                t = wfpool.tile([KT_ROWS[kt], NKPL * 128], BF16,
                                name="t", tag=f"wf{kt}")
                nc.sync.dma_start(
                    t[:], wfT[r0:r0 + KT_ROWS[kt], c0:c0 + NKPL * 128])
                wf_t.append(t)
                r0 += KT_ROWS[kt]
            return wf_t

        def emit_conv_f(ct, wf_t):
            """final conv for one channel tile -> wgt (128, 9, 196) bf16.
            Two taps share one psum bank; drained in 2-tap batches."""
            wgt = wgtpool.tile([128, NKPL, NIJ], BF16, name="t", tag="wgt")
            for t0 in range(0, NKPL, 2):
                ntap = min(2, NKPL - t0)
                u = ps_u.tile([128, NB2], F32, name="t", tag="u")
                for sub in range(ntap):
                    kpl = t0 + sub
                    dst = u[:, NIJ * sub:NIJ * (sub + 1)]
                    for kt in range(5):
                        nc.tensor.matmul(
                            dst,
                            wf_t[kt][:, 128 * kpl:128 * (kpl + 1)],
                            p3[kt][:],
                            start=(kt == 0), stop=(kt == 4))
                nc.scalar.activation(
                    wgt[:, t0:t0 + ntap, :],
                    u[:, 0:NIJ * ntap].rearrange("p (t f) -> p t f", t=ntap),
                    AF.Copy)
            return wgt

        def emit_taps(ct, wgt, pst, preadds):
            """9 tap products (DVE+Pool), pre-adds, identity-sum into pst."""
            prods = {}
            for t in range(NKPL):
                di, dj = t // 3, t % 3
                wgb = (wgt[:, t, :].rearrange("p (i j) -> p i j", i=HP, j=HP)
                       .unsqueeze(1).broadcast_to((128, BPC, HP, HP)))
                hwin = h_sb[ct][:, :, di:di + HP, dj:dj + HP]
                prod = prodpool.tile([128, BPC * NIJ], BF16,
                                     name="t", tag=f"prod{t}")
                pr = prod[:].rearrange("p (b i j) -> p b i j",
                                       b=BPC, i=HP, j=HP)
                eng = nc.gpsimd if t in POOL_TAPS else nc.vector
                eng.tensor_mul(pr, hwin, wgb)
                prods[t] = prod

            merged = set()
            for (a, b) in preadds:
                nc.vector.tensor_add(prods[a][:], prods[a][:], prods[b][:])
                merged.add(b)
            streams = [prods[t] for t in range(NKPL) if t not in merged]

            for si, prod in enumerate(streams):
                for ch in range(NCHUNK):
                    nc.tensor.matmul(
                        pst[:, ch, 0:NB2], ident[:],
                        prod[:, NB2 * ch:NB2 * (ch + 1)],
                        start=(si == 0), stop=(si == len(streams) - 1))

        # =================== program ===================
        ps0 = ps_u.tile([INTER, NIJ], F32, name="t", tag="u")
        conv3x3(w0_sb, posi_sb, ps0[:])
        layernorm_relu(ps0[:], gb_sb[:, 0, :], gb_sb[:, 1, :], pad1)

        # proj_in prologue interleaved with the LN chain
        emit_proj_in(CT_ORDER[0])

        ps1 = ps_u.tile([INTER, NIJ], F32, name="t", tag="u")
        conv3x3(w1_sb, pad1, ps1[:])
        layernorm_relu(ps1[:], gb_sb[:, 2, :], gb_sb[:, 3, :], pad2)

        emit_proj_in(CT_ORDER[1])

        ps2 = ps_u.tile([INTER, NIJ], F32, name="t", tag="u")
        conv3x3(w2_sb, pad2, ps2[:])
        layernorm_relu(ps2[:], gb_sb[:, 4, :], gb_sb[:, 5, :], pad3)

        # warm the gelu table now (off the critical path)
        nc.scalar.activation(wsink[:], warm[:], AF.Gelu)

        # im2col of pad3 for the final conv (576 contraction rows)
        qengs = [nc.sync, nc.scalar]
        for kt in range(5):
            nk = KT_ROWS[kt] // 64
            for sub in range(nk):
                kap = 2 * kt + sub
                di, dj = kap // 3, kap % 3
                srcw = pad3[:, di:di + HP, dj:dj + HP]
                dst = p3[kt][64 * sub:64 * (sub + 1), :]
                dst = dst.rearrange("p (i j) -> p i j", i=HP, j=HP)
                qengs[kap % 2].dma_start(dst, srcw)

        emit_proj_in(CT_ORDER[2])

        # conv-f runs one stage ahead of the tap loop so wgt is drained
        # before the stage's first DVE mult
        wf0 = emit_wf_load(CT_ORDER[0])
        wgt_next = emit_conv_f(CT_ORDER[0], wf0)
        wf_next = emit_wf_load(CT_ORDER[1])
        ga_cur = None
        for s, ct in enumerate(CT_ORDER):
            wgt = wgt_next
            if s + 1 < NCT:
                wgt_next = emit_conv_f(CT_ORDER[s + 1], wf_next)
            if s + 2 < NCT:
                wf_next = emit_wf_load(CT_ORDER[s + 2])
            if s + 3 < NCT:
                emit_proj_in(CT_ORDER[s + 3])

            pst = ps_pst.tile([128, NCHUNK, 512], F32, name="t", tag="pst")
            preadds = PRE_A3 if s % 2 == 0 else PRE_A4
            emit_taps(ct, wgt, pst, preadds)

            if s % 2 == 0:
                # x1 tile: gelu straight from psum
                ga = gapool.tile([128, BPC * NIJ], BF16, name="t", tag="ga")
                nc.scalar.activation(
                    ga[:].rearrange("p (c f) -> p c f", c=NCHUNK),
                    pst[:, :, 0:NB2], AF.Gelu)
                ga_cur = ga
            else:
                # x2 tile: gated = gelu(x1) * x2  (psum f32 * sbuf bf16)
                pair = ct - NGT
                nc.vector.tensor_mul(
                    gated[pair][:].rearrange("p (c f) -> p c f", c=NCHUNK),
                    pst[:, :, 0:NB2],
                    ga_cur[:].rearrange("p (c f) -> p c f", c=NCHUNK))

        # ---------------- proj_out: W_out @ gated ----------------
        outpool = ctx.enter_context(tc.tile_pool(name="outpool", bufs=4))
        for m in range(2):
            for ch in range(NCHUNK):
                u = ps_u.tile([128, NB2], F32, name="t", tag="u")
                for kt in range(NGT):
                    nc.tensor.matmul(
                        u[:],
                        wout_sb[kt][:, 128 * m:128 * (m + 1)],
                        gated[kt][:, NB2 * ch:NB2 * (ch + 1)],
                        start=(kt == 0), stop=(kt == NGT - 1))
                ot = outpool.tile([128, NB2], F32, name="t", tag="ot")
                nc.scalar.activation(ot[:], u[:], AF.Copy)
                nc.sync.dma_start(
                    out_f[128 * m:128 * (m + 1), NB2 * ch:NB2 * (ch + 1)],
                    ot[:])

    nc.compile()
    return nc


def _pack_shared(inputs):
    """Pack the batch-independent tensors (host-side layout marshalling)."""
    W_in = np.asarray(inputs["W_in"], np.float32)
    W_out = np.asarray(inputs["W_out"], np.float32)
    posi = np.asarray(inputs["posi_map"], np.float32)
    w0 = np.asarray(inputs["w0"], np.float32)
    w1 = np.asarray(inputs["w1"], np.float32)
    w2 = np.asarray(inputs["w2"], np.float32)
    wf = np.asarray(inputs["wf"], np.float32)

    padc = np.arange(CH)
    padc = np.where(padc < HID, padc, padc + (HIDP - HID))

    winP = np.zeros((CHP, DIM), np.float32)
    winP[padc] = W_in
    winT = np.ascontiguousarray(winP.T).astype(ml_dtypes.bfloat16)

    w0T = np.ascontiguousarray(
        w0.transpose(1, 2, 3, 0).reshape(4, 9, INTER)).astype(ml_dtypes.bfloat16)
    w1T = np.ascontiguousarray(
        w1.transpose(1, 2, 3, 0).reshape(INTER, 9, INTER)
    ).astype(ml_dtypes.bfloat16)
    w2T = np.ascontiguousarray(
        w2.transpose(1, 2, 3, 0).reshape(INTER, 9, INTER)
    ).astype(ml_dtypes.bfloat16)

    posiP = np.zeros((4, PH, PH), np.float32)
    posiP[:, 1:15, 1:15] = posi[0]
    posiP = posiP.astype(ml_dtypes.bfloat16)

    gbs = [np.asarray(inputs[k], np.float32).reshape(INTER, NIJ)
           for k in ("g0", "b0", "g1", "b1", "g2", "b2")]
    gb = np.stack(gbs, axis=1)   # (64, 6, 196)

    # wfT[(kh,kw,cin) row, ct*1152 + kpl*128 + p] = wf[c*9+kpl, cin, kh, kw]
    wf5 = wf.reshape(CH, NKPL, INTER, 3, 3)
    wf5 = wf5.transpose(3, 4, 2, 1, 0)          # (kh, kw, cin, kpl, c)
    wfTp = np.zeros((576, NKPL, CHP), np.float32)
    wfTp[:, :, padc] = wf5.reshape(576, NKPL, CH)
    wfTp = wfTp.reshape(576, NKPL, NCT, 128).transpose(0, 2, 1, 3)
    wfT = np.ascontiguousarray(
        wfTp.reshape(576, NKPL * CHP)).astype(ml_dtypes.bfloat16)

    woP = np.zeros((HIDP, DIM), np.float32)
    woP[:HID] = W_out.T
    woutT = woP.astype(ml_dtypes.bfloat16)

    return dict(winT=winT, posiP=posiP, w0T=w0T, w1T=w1T, w2T=w2T,
                gb=np.ascontiguousarray(gb), wfT=wfT, woutT=woutT,
                identD=np.eye(128, dtype=ml_dtypes.bfloat16))


def kernel(**inputs) -> np.ndarray:
    if "nc" not in _CACHE:
        _CACHE["nc"] = _build_nc()
    nc = _CACHE["nc"]

    x = np.asarray(inputs["x"], np.float32)     # (64, 256, 14, 14)
    shared = _pack_shared(inputs)

    in_maps = []
    for c in range(NCORES):
        xc = x[BPC * c:BPC * (c + 1)]           # (8, 256, 14, 14)
        xT = np.ascontiguousarray(
            xc.transpose(1, 0, 2, 3).reshape(DIM, BPC * NIJ)
        ).astype(ml_dtypes.bfloat16)
        m = dict(shared)
        m["xT"] = xT
        in_maps.append(m)

    res = run_bass_kernel_spmd(nc, in_maps, list(range(NCORES)))
    outs = []
    for c in range(NCORES):
        o = res.results[c]["out_f"].reshape(DIM, BPC, HP, HP)
        outs.append(o.transpose(1, 0, 2, 3))
    return np.ascontiguousarray(np.concatenate(outs, axis=0), dtype=np.float32)
